# revision 4
# baseline (speedup 1.0000x reference)
"""Trainium2 Bass kernel for single-head 2D attention (B=16, C=512, H=W=32).

Data-parallel over batch: 16 batch items / 8 cores = 2 per core. Weights
replicated. All matmuls run in float32r (full PE rate); layouts are chosen
so no on-device transpose is ever needed:

  per batch item b (x_cn = x[b] viewed as [C, N=1024], channel-major):
    Qt[o,n] = sum_c wqT[c,o] x[c,n] + bq[o]      (lhsT=wqT, rhs=x)
    Kt[o,n] = likewise
    V[n,o]  = sum_c x[c,n] wvT[c,o]              (lhsT=x,   rhs=wvT)
    St[j,i] = sum_o Kt[o,j] Qt[o,i]              (lhsT=Kt,  rhs=Qt)
    E[j,i]  = exp(St[j,i] / sqrt(C))             (ACT, no max-subtract:
                                                  scores are O(+-6))
    den[*,i]= sum_j E[j,i]   via all-ones lhsT   (sum over partitions AND
                                                  broadcast to 128 parts)
    svT[c,i]= (sum_j V[j,c] E[j,i]) * recip[i]   (lhsT=V, rhs=E)
    y[c',n] = x[c',n] + sum_c woT[c,c'] svT[c,n] + bo_eff[c']
  with bo_eff = bo + wo @ bv (V bias folded in on the host).
"""

import math

import numpy as np

import concourse.bass as bass
import concourse.mybir as mybir
import concourse.tile as tile
from concourse import bacc, bass_utils

B, C, H, W = 16, 512, 32, 32
N = H * W           # 1024 tokens
NCORES = 8
BPC = B // NCORES   # batch items per core
P = 128
CO = C // P         # 4 channel chunks
NB = N // 512       # 2 psum-bank slices of the token dim
NT = N // P         # 8 token chunks

_CACHE: dict = {}


def _build(reps: int = 1):
    f32 = mybir.dt.float32
    f32r = mybir.dt.float32r
    Ident = mybir.ActivationFunctionType.Identity
    Exp = mybir.ActivationFunctionType.Exp
    add = mybir.AluOpType.add

    nc = bacc.Bacc("TRN2", debug=False, enable_asserts=False, num_devices=NCORES)
    x_d = nc.dram_tensor("x", (BPC, C, N), f32r, kind="ExternalInput").ap()
    w_d = {
        k: nc.dram_tensor(f"w{k}t", (C, C), f32r, kind="ExternalInput").ap()
        for k in ("q", "k", "v", "o")
    }
    bq_d = nc.dram_tensor("bq", (P, CO), f32, kind="ExternalInput").ap()
    bk_d = nc.dram_tensor("bk", (P, CO), f32, kind="ExternalInput").ap()
    bo_d = nc.dram_tensor("bo", (P, CO), f32, kind="ExternalInput").ap()
    ones_d = nc.dram_tensor("ones", (P, P), f32r, kind="ExternalInput").ap()
    y_d = nc.dram_tensor("y", (BPC, C, N), f32, kind="ExternalOutput").ap()

    with tile.TileContext(nc) as tc:
        with (
            tc.tile_pool(name="wp", bufs=1) as wp,
            tc.tile_pool(name="xp", bufs=2) as xp,
            tc.tile_pool(name="qkp", bufs=1) as qkp,
            tc.tile_pool(name="vp", bufs=2) as vp,
            tc.tile_pool(name="ep", bufs=1) as ep,
            tc.tile_pool(name="svp", bufs=1) as svp,
            tc.tile_pool(name="rp", bufs=1) as rp,
            tc.tile_pool(name="yp", bufs=4) as yp,
            tc.tile_pool(name="ps", bufs=4, space="PSUM") as ps,
        ):
            # Per-ci chunked loads in first-consumption order, so the PE can
            # start on (wq ci-chunk 0, x ci-chunk 0) instead of waiting for
            # the full 6 MB weight+activation prefix.
            wt = {
                k: wp.tile([P, CO, C], f32r, tag=f"w{k}", name=f"w{k}")
                for k in ("q", "k", "v", "o")
            }
            x_tiles = [
                xp.tile([P, CO, N], f32r, tag="x", name=f"x{b}") for b in range(BPC)
            ]
            w_r = {
                k: w_d[k].rearrange("(co p) o -> p co o", p=P) for k in ("q", "k", "v", "o")
            }
            x_r = [x_d[b].rearrange("(ci p) n -> p ci n", p=P) for b in range(BPC)]
            for ci in range(CO):
                nc.sync.dma_start(wt["q"][:, ci], w_r["q"][:, ci])
                nc.sync.dma_start(x_tiles[0][:, ci], x_r[0][:, ci])
            bq_t = wp.tile([P, CO], f32, tag="bq")
            nc.sync.dma_start(bq_t[:], bq_d)
            for ci in range(CO):
                nc.sync.dma_start(wt["k"][:, ci], w_r["k"][:, ci])
            bk_t = wp.tile([P, CO], f32, tag="bk")
            nc.sync.dma_start(bk_t[:], bk_d)
            for ci in range(CO):
                nc.sync.dma_start(wt["v"][:, ci], w_r["v"][:, ci])
            ones_t = wp.tile([P, P], f32r, tag="ones")
            nc.sync.dma_start(ones_t[:], ones_d)
            for ci in range(CO):
                nc.sync.dma_start(wt["o"][:, ci], w_r["o"][:, ci])
            bo_t = wp.tile([P, CO], f32, tag="bo")
            nc.sync.dma_start(bo_t[:], bo_d)
            for ci in range(CO):
                nc.sync.dma_start(x_tiles[1][:, ci], x_r[1][:, ci])

            inv_sqrt_c = 1.0 / math.sqrt(C)

            for b in [i for _ in range(reps) for i in range(BPC)]:
                x_sb = x_tiles[b]

                # --- Qt / Kt projections (channel-major) ---
                qt = qkp.tile([P, CO, N], f32r, tag="qt")
                kt = qkp.tile([P, CO, N], f32r, tag="kt")
                for t_sb, w_t, b_t in ((qt, wt["q"], bq_t), (kt, wt["k"], bk_t)):
                    for oc in range(CO):
                        for nb in range(NB):
                            pt = ps.tile([P, 512], f32, tag="ps")
                            for ci in range(CO):
                                nc.tensor.matmul(
                                    pt[:],
                                    w_t[:, ci, oc * P:(oc + 1) * P],
                                    x_sb[:, ci, nb * 512:(nb + 1) * 512],
                                    start=(ci == 0), stop=(ci == CO - 1),
                                )
                            nc.scalar.activation(
                                t_sb[:, oc, nb * 512:(nb + 1) * 512], pt[:],
                                Ident, bias=b_t[:, oc:oc + 1],
                            )

                # --- V projection (token-major, bias folded into bo_eff) ---
                v_sb = vp.tile([P, NT, C], f32r, tag="v")
                for t8 in range(NT):
                    pt = ps.tile([P, 512], f32, tag="ps")
                    for ci in range(CO):
                        nc.tensor.matmul(
                            pt[:],
                            x_sb[:, ci, t8 * P:(t8 + 1) * P],
                            wt["v"][:, ci, :],
                            start=(ci == 0), stop=(ci == CO - 1),
                        )
                    nc.vector.tensor_copy(v_sb[:, t8, :], pt[:])

                # --- St = Kt^T Qt, then exp (scores scaled inside ACT) ---
                est = ep.tile([P, NT, N], f32r, tag="est")
                for jc in range(NT):
                    for ib in range(NB):
                        pt = ps.tile([P, 512], f32, tag="ps")
                        for oc in range(CO):
                            nc.tensor.matmul(
                                pt[:],
                                kt[:, oc, jc * P:(jc + 1) * P],
                                qt[:, oc, ib * 512:(ib + 1) * 512],
                                start=(oc == 0), stop=(oc == CO - 1),
                            )
                        nc.scalar.activation(
                            est[:, jc, ib * 512:(ib + 1) * 512], pt[:],
                            Exp, scale=inv_sqrt_c,
                        )

                # --- softmax denominators: all-ones lhsT sums over partitions
                #     and broadcasts the result to every partition ---
                recip = rp.tile([P, N], f32, tag="recip")
                for ib in range(NB):
                    pt = ps.tile([P, 512], f32, tag="ps")
                    for jc in range(NT):
                        nc.tensor.matmul(
                            pt[:], ones_t[:],
                            est[:, jc, ib * 512:(ib + 1) * 512],
                            start=(jc == 0), stop=(jc == NT - 1),
                        )
                    nc.vector.reciprocal(recip[:, ib * 512:(ib + 1) * 512], pt[:])

                # --- svT = (V^T E) * recip  (channel-major) ---
                sv = svp.tile([P, CO, N], f32r, tag="sv")
                for cc in range(CO):
                    for ib in range(NB):
                        pt = ps.tile([P, 512], f32, tag="ps")
                        for jc in range(NT):
                            nc.tensor.matmul(
                                pt[:],
                                v_sb[:, jc, cc * P:(cc + 1) * P],
                                est[:, jc, ib * 512:(ib + 1) * 512],
                                start=(jc == 0), stop=(jc == NT - 1),
                            )
                        nc.vector.tensor_mul(
                            sv[:, cc, ib * 512:(ib + 1) * 512], pt[:],
                            recip[:, ib * 512:(ib + 1) * 512],
                        )

                # --- output projection + bias + residual ---
                for c2 in range(CO):
                    for nb in range(NB):
                        pt = ps.tile([P, 512], f32, tag="ps")
                        for cc in range(CO):
                            nc.tensor.matmul(
                                pt[:],
                                wt["o"][:, cc, c2 * P:(c2 + 1) * P],
                                sv[:, cc, nb * 512:(nb + 1) * 512],
                                start=(cc == 0), stop=(cc == CO - 1),
                            )
                        yt = yp.tile([P, 512], f32, tag="y")
                        nc.vector.scalar_tensor_tensor(
                            yt[:], pt[:], bo_t[:, c2:c2 + 1],
                            x_sb[:, c2, nb * 512:(nb + 1) * 512].bitcast(f32),
                            add, add,
                        )
                        nc.sync.dma_start(
                            y_d[b, c2 * P:(c2 + 1) * P, nb * 512:(nb + 1) * 512],
                            yt[:],
                        )
    nc.compile()
    return nc


def _prep_inputs(inputs):
    x = np.asarray(inputs["x"], np.float32).reshape(B, C, N)
    wts = {}
    for k in ("q", "k", "v", "o"):
        wts[f"w{k}t"] = np.ascontiguousarray(np.asarray(inputs[f"w{k}"], np.float32).T)
    bq = np.asarray(inputs["bq"], np.float32)
    bk = np.asarray(inputs["bk"], np.float32)
    bv = np.asarray(inputs["bv"], np.float32)
    bo = np.asarray(inputs["bo"], np.float32)
    wo = np.asarray(inputs["wo"], np.float32)
    bo_eff = bo + wo @ bv

    def per_part(v):  # [C] -> [P, CO]
        return np.ascontiguousarray(v.reshape(CO, P).T)

    shared = {
        **wts,
        "bq": per_part(bq),
        "bk": per_part(bk),
        "bo": per_part(bo_eff),
        "ones": np.ones((P, P), np.float32),
    }
    in_maps = [
        {**shared, "x": np.ascontiguousarray(x[i * BPC:(i + 1) * BPC])}
        for i in range(NCORES)
    ]
    return in_maps


def kernel(**inputs) -> np.ndarray:
    if "nc" not in _CACHE:
        _CACHE["nc"] = _build()
    nc = _CACHE["nc"]
    in_maps = _prep_inputs(inputs)
    res = bass_utils.run_bass_kernel_spmd(nc, in_maps, core_ids=list(range(NCORES)))
    y = np.concatenate([r["y"] for r in res.results], axis=0)
    return y.reshape(B, C, H, W)


# revision 17
# speedup vs baseline: 29558.8118x; 29558.8118x over previous
"""Trainium2 Bass kernel for single-head 2D attention (B=16, C=512, H=W=32).

Data-parallel over batch: 16 batch items / 8 cores = 2 per core. Weights
replicated. All matmuls run in float32r (full PE rate); layouts are chosen
so no on-device transpose is ever needed:

  per batch item b (x_cn = x[b] viewed as [C, N=1024], channel-major):
    Qt[o,n] = sum_c wqT[c,o] x[c,n] + bq[o]      (lhsT=wqT, rhs=x)
    Kt[o,n] = likewise
    V[n,o]  = sum_c x[c,n] wvT[c,o]              (lhsT=x,   rhs=wvT)
    St[j,i] = sum_o Kt[o,j] Qt[o,i]              (lhsT=Kt,  rhs=Qt)
    E[j,i]  = exp(St[j,i] / sqrt(C))             (ACT, no max-subtract:
                                                  scores are O(+-6))
    den[*,i]= sum_j E[j,i]   via all-ones lhsT   (sum over partitions AND
                                                  broadcast to 128 parts)
    svT[c,i]= (sum_j V[j,c] E[j,i]) * recip[i]   (lhsT=V, rhs=E)
    y[c',n] = x[c',n] + sum_c woT[c,c'] svT[c,n] + bo_eff[c']
  with bo_eff = bo + wo @ bv (V bias folded in on the host).
"""

import math

import numpy as np

import concourse.mybir as mybir
import concourse.tile as tile
from concourse import bacc, bass_utils

B, C, H, W = 16, 512, 32, 32
N = H * W           # 1024 tokens
NCORES = 8
BPC = B // NCORES   # batch items per core
P = 128
CO = C // P         # 4 channel chunks
NB = N // 512       # 2 psum-bank slices of the token dim
NT = N // P         # 8 token chunks

_CACHE: dict = {}


def _build(reps: int = 1):
    f32 = mybir.dt.float32
    f32r = mybir.dt.float32r
    Ident = mybir.ActivationFunctionType.Identity
    Exp = mybir.ActivationFunctionType.Exp
    add = mybir.AluOpType.add

    nc = bacc.Bacc("TRN2", debug=False, enable_asserts=False, num_devices=NCORES)
    x_d = nc.dram_tensor("x", (BPC, C, N), f32r, kind="ExternalInput").ap()
    w_d = {
        k: nc.dram_tensor(f"w{k}t", (C, C), f32r, kind="ExternalInput").ap()
        for k in ("q", "k", "v", "o")
    }
    bq_d = nc.dram_tensor("bq", (P, CO), f32, kind="ExternalInput").ap()
    bk_d = nc.dram_tensor("bk", (P, CO), f32, kind="ExternalInput").ap()
    bo_d = nc.dram_tensor("bo", (P, CO), f32, kind="ExternalInput").ap()
    ones_d = nc.dram_tensor("ones", (P, P), f32r, kind="ExternalInput").ap()
    y_d = nc.dram_tensor("y", (BPC, C, N), f32, kind="ExternalOutput").ap()

    with tile.TileContext(nc) as tc:
        with (
            tc.tile_pool(name="wp", bufs=1) as wp,
            tc.tile_pool(name="xp", bufs=2) as xp,
            tc.tile_pool(name="qkp", bufs=1) as qkp,
            tc.tile_pool(name="vp", bufs=2) as vp,
            tc.tile_pool(name="ep", bufs=1) as ep,
            tc.tile_pool(name="svp", bufs=1) as svp,
            tc.tile_pool(name="rp", bufs=1) as rp,
            tc.tile_pool(name="yp", bufs=4) as yp,
            tc.tile_pool(name="ps", bufs=6, space="PSUM") as ps,
        ):
            # Per-ci chunked loads in first-consumption order, so the PE can
            # start on (wq ci-chunk 0, x ci-chunk 0) instead of waiting for
            # the full 6 MB weight+activation prefix.
            wt = {
                k: wp.tile([P, CO, C], f32r, tag=f"w{k}", name=f"w{k}")
                for k in ("q", "k", "v", "o")
            }
            x_tiles = [
                xp.tile([P, CO, N], f32r, tag="x", name=f"x{b}") for b in range(BPC)
            ]
            w_r = {
                k: w_d[k].rearrange("(co p) o -> p co o", p=P) for k in ("q", "k", "v", "o")
            }
            x_r = [x_d[b].rearrange("(ci p) n -> p ci n", p=P) for b in range(BPC)]
            # PE warm-up on a memset tile (no DMA dependency): keeps the HAM
            # activity window busy while the first wq/x chunks stream in.
            warm_t = wp.tile([P, P], f32, tag="warm_t")
            nc.vector.memset(warm_t[:], 0.0)
            with tc.tile_pool(name="warm", bufs=1, space="PSUM") as warmp:
                for i in range(8):
                    wpt = warmp.tile([P, P], f32, tag="warm", name=f"warm{i}")
                    nc.tensor.matmul(wpt[:], warm_t[:], warm_t[:],
                                     start=True, stop=True)
            # loads in first-consumption order
            for ci in range(CO):
                nc.sync.dma_start(wt["q"][:, ci], w_r["q"][:, ci])
                nc.sync.dma_start(x_tiles[0][:, ci, 0:512], x_r[0][:, ci, 0:512])
            bq_t = wp.tile([P, CO], f32, tag="bq")
            nc.sync.dma_start(bq_t[:], bq_d)
            bk_t = wp.tile([P, CO], f32, tag="bk")
            nc.sync.dma_start(bk_t[:], bk_d)
            for ci in range(CO):
                nc.sync.dma_start(wt["k"][:, ci], w_r["k"][:, ci])
            for ci in range(CO):
                nc.sync.dma_start(x_tiles[0][:, ci, 512:1024], x_r[0][:, ci, 512:1024])
            for ci in range(CO):
                nc.sync.dma_start(wt["v"][:, ci], w_r["v"][:, ci])
            ones_t = wp.tile([P, P], f32r, tag="ones")
            nc.sync.dma_start(ones_t[:], ones_d)
            for ci in range(CO):
                nc.sync.dma_start(wt["o"][:, ci], w_r["o"][:, ci])
            bo_t = wp.tile([P, CO], f32, tag="bo")
            nc.sync.dma_start(bo_t[:], bo_d)
            for ci in range(CO):
                nc.sync.dma_start(x_tiles[1][:, ci], x_r[1][:, ci])

            inv_sqrt_c = 1.0 / math.sqrt(C)

            for b in [i for _ in range(reps) for i in range(BPC)]:
                x_sb = x_tiles[b]

                # --- Qt / Kt projections (channel-major) ---
                qt = qkp.tile([P, CO, N], f32r, tag="qt")
                kt = qkp.tile([P, CO, N], f32r, tag="kt")
                for nb in range(NB):
                    for t_sb, w_t, b_t in ((qt, wt["q"], bq_t), (kt, wt["k"], bk_t)):
                        for oc in range(CO):
                            pt = ps.tile([P, 512], f32, tag="ps")
                            for ci in range(CO):
                                nc.tensor.matmul(
                                    pt[:],
                                    w_t[:, ci, oc * P:(oc + 1) * P],
                                    x_sb[:, ci, nb * 512:(nb + 1) * 512],
                                    start=(ci == 0), stop=(ci == CO - 1),
                                )
                            nc.scalar.activation(
                                t_sb[:, oc, nb * 512:(nb + 1) * 512], pt[:],
                                Ident, bias=b_t[:, oc:oc + 1],
                            )

                # --- V projection (token-major, bias folded into bo_eff) ---
                v_sb = vp.tile([P, NT, C], f32r, tag="v")
                for t8 in range(NT):
                    pt = ps.tile([P, 512], f32, tag="ps")
                    for ci in range(CO):
                        nc.tensor.matmul(
                            pt[:],
                            x_sb[:, ci, t8 * P:(t8 + 1) * P],
                            wt["v"][:, ci, :],
                            start=(ci == 0), stop=(ci == CO - 1),
                        )
                    nc.vector.tensor_copy(v_sb[:, t8, :], pt[:])

                # --- St = Kt^T Qt, then exp (scores scaled inside ACT).
                #     DVE accumulates the softmax denominators chunk by
                #     chunk as the exps land (frees the PE). ---
                est = ep.tile([P, NT, N], f32r, tag="est")
                esum = rp.tile([P, N], f32r, tag="esum")
                for jc in range(NT):
                    for ib in range(NB):
                        pt = ps.tile([P, 512], f32, tag="ps")
                        for oc in range(CO):
                            nc.tensor.matmul(
                                pt[:],
                                kt[:, oc, jc * P:(jc + 1) * P],
                                qt[:, oc, ib * 512:(ib + 1) * 512],
                                start=(oc == 0), stop=(oc == CO - 1),
                            )
                        nc.scalar.activation(
                            est[:, jc, ib * 512:(ib + 1) * 512], pt[:],
                            Exp, scale=inv_sqrt_c,
                        )
                    if jc == 1:
                        nc.vector.tensor_add(esum[:], est[:, 0, :], est[:, 1, :])
                    elif jc > 1:
                        nc.vector.tensor_add(esum[:], esum[:], est[:, jc, :])

                # --- svT = (V^T E) * recip  (channel-major). The partition
                #     sum+broadcast of esum (all-ones matmul) and reciprocal
                #     are slotted after the first group's matmuls so the PE
                #     never waits on the DVE chain. ---
                recip = rp.tile([P, N], f32, tag="recip")
                sv = svp.tile([P, CO, N], f32r, tag="sv")
                first = True
                for cc in range(CO):
                    for ib in range(NB):
                        pt = ps.tile([P, 512], f32, tag="ps")
                        for jc in range(NT):
                            nc.tensor.matmul(
                                pt[:],
                                v_sb[:, jc, cc * P:(cc + 1) * P],
                                est[:, jc, ib * 512:(ib + 1) * 512],
                                start=(jc == 0), stop=(jc == NT - 1),
                            )
                        if first:
                            first = False
                            for db in range(NB):
                                dpt = ps.tile([P, 512], f32, tag="ps")
                                nc.tensor.matmul(
                                    dpt[:], ones_t[:],
                                    esum[:, db * 512:(db + 1) * 512],
                                    start=True, stop=True,
                                )
                                nc.vector.reciprocal(
                                    recip[:, db * 512:(db + 1) * 512], dpt[:])
                        nc.vector.tensor_mul(
                            sv[:, cc, ib * 512:(ib + 1) * 512], pt[:],
                            recip[:, ib * 512:(ib + 1) * 512],
                        )

                # --- output projection + bias + residual ---
                for c2 in range(CO):
                    for nb in range(NB):
                        pt = ps.tile([P, 512], f32, tag="ps")
                        for cc in range(CO):
                            nc.tensor.matmul(
                                pt[:],
                                wt["o"][:, cc, c2 * P:(c2 + 1) * P],
                                sv[:, cc, nb * 512:(nb + 1) * 512],
                                start=(cc == 0), stop=(cc == CO - 1),
                            )
                        yt = yp.tile([P, 512], f32, tag="y")
                        nc.vector.scalar_tensor_tensor(
                            yt[:], pt[:], bo_t[:, c2:c2 + 1],
                            x_sb[:, c2, nb * 512:(nb + 1) * 512].bitcast(f32),
                            add, add,
                        )
                        nc.sync.dma_start(
                            y_d[b, c2 * P:(c2 + 1) * P, nb * 512:(nb + 1) * 512],
                            yt[:],
                        )
    nc.compile()
    return nc


def _prep_inputs(inputs):
    x = np.asarray(inputs["x"], np.float32).reshape(B, C, N)
    wts = {}
    for k in ("q", "k", "v", "o"):
        wts[f"w{k}t"] = np.ascontiguousarray(np.asarray(inputs[f"w{k}"], np.float32).T)
    bq = np.asarray(inputs["bq"], np.float32)
    bk = np.asarray(inputs["bk"], np.float32)
    bv = np.asarray(inputs["bv"], np.float32)
    bo = np.asarray(inputs["bo"], np.float32)
    wo = np.asarray(inputs["wo"], np.float32)
    bo_eff = bo + wo @ bv

    def per_part(v):  # [C] -> [P, CO]
        return np.ascontiguousarray(v.reshape(CO, P).T)

    shared = {
        **wts,
        "bq": per_part(bq),
        "bk": per_part(bk),
        "bo": per_part(bo_eff),
        "ones": np.ones((P, P), np.float32),
    }
    in_maps = [
        {**shared, "x": np.ascontiguousarray(x[i * BPC:(i + 1) * BPC])}
        for i in range(NCORES)
    ]
    return in_maps


def _make_axon_runner(nc):
    """Cached jitted shard_map runner for the axon/PJRT path, so repeated
    kernel() calls execute without re-tracing (the stock
    run_bass_kernel_spmd path builds a fresh jit closure per call)."""
    import jax
    from jax.sharding import Mesh, NamedSharding, PartitionSpec

    import warnings

    with warnings.catch_warnings():
        warnings.simplefilter("ignore")
        from jax.experimental.shard_map import shard_map

    import concourse.bass2jax as b2j

    b2j.install_neuronx_cc_hook()
    partition_name = nc.partition_id_tensor.name if nc.partition_id_tensor else None
    in_names, out_names, out_avals = [], [], []
    for alloc in nc.m.functions[0].allocations:
        if not isinstance(alloc, mybir.MemoryLocationSet):
            continue
        name = alloc.memorylocations[0].name
        if alloc.kind == "ExternalInput":
            if name != partition_name:
                in_names.append(name)
        elif alloc.kind == "ExternalOutput":
            out_names.append(name)
            out_avals.append(
                jax.core.ShapedArray(tuple(alloc.tensor_shape),
                                     mybir.dt.np(alloc.dtype)))
    n_params = len(in_names)
    bind_in_names = list(in_names) + list(out_names)
    if partition_name is not None:
        bind_in_names.append(partition_name)

    def _body(*args):
        operands = list(args)
        if partition_name is not None:
            operands.append(b2j.partition_id_tensor())
        return tuple(b2j._bass_exec_p.bind(
            *operands,
            out_avals=tuple(out_avals),
            in_names=tuple(bind_in_names),
            out_names=tuple(out_names),
            lowering_input_output_aliases=(),
            sim_require_finite=True,
            sim_require_nnan=True,
            nc=nc,
        ))

    devices = jax.devices()[:NCORES]
    mesh = Mesh(np.asarray(devices), ("core",))
    n_outs = len(out_avals)
    fn = jax.jit(
        shard_map(_body, mesh=mesh,
                  in_specs=(PartitionSpec("core"),) * (n_params + n_outs),
                  out_specs=(PartitionSpec("core"),) * n_outs,
                  check_rep=False),
        keep_unused=True,
    )
    sharding = NamedSharding(mesh, PartitionSpec("core"))
    zeros = [
        np.zeros((NCORES * a.shape[0], *a.shape[1:]), a.dtype) for a in out_avals
    ]
    dev_zeros = [jax.device_put(z, sharding) for z in zeros]

    def run(in_maps):
        concat_in = [
            np.concatenate([np.asarray(m[nm]) for m in in_maps], axis=0)
            for nm in in_names
        ]
        dev_in = [jax.device_put(a, sharding) for a in concat_in]
        outs = fn(*dev_in, *dev_zeros)
        return [
            {nm: np.asarray(outs[i]).reshape(NCORES, *out_avals[i].shape)[c]
             for i, nm in enumerate(out_names)}
            for c in range(NCORES)
        ]

    return run


def kernel(**inputs) -> np.ndarray:
    if "nc" not in _CACHE:
        _CACHE["nc"] = _build()
    nc = _CACHE["nc"]
    in_maps = _prep_inputs(inputs)

    from concourse._compat import axon_active

    if axon_active():
        if "runner" not in _CACHE:
            _CACHE["runner"] = _make_axon_runner(nc)
        results = _CACHE["runner"](in_maps)
    else:
        results = bass_utils.run_bass_kernel_spmd(
            nc, in_maps, core_ids=list(range(NCORES))).results
    y = np.concatenate([r["y"] for r in results], axis=0)
    return y.reshape(B, C, H, W)


# revision 18
# speedup vs baseline: 30262.8551x; 1.0238x over previous
"""Trainium2 Bass kernel for single-head 2D attention (B=16, C=512, H=W=32).

Data-parallel over batch: 16 batch items / 8 cores = 2 per core. Weights
replicated. All matmuls run in float32r (full PE rate); layouts are chosen
so no on-device transpose is ever needed:

  per batch item b (x_cn = x[b] viewed as [C, N=1024], channel-major):
    Qt[o,n] = sum_c wqT[c,o] x[c,n] + bq[o]      (lhsT=wqT, rhs=x)
    Kt[o,n] = likewise
    V[n,o]  = sum_c x[c,n] wvT[c,o]              (lhsT=x,   rhs=wvT)
    St[j,i] = sum_o Kt[o,j] Qt[o,i]              (lhsT=Kt,  rhs=Qt)
    E[j,i]  = exp(St[j,i] / sqrt(C))             (ACT, no max-subtract:
                                                  scores are O(+-6))
    den[*,i]= sum_j E[j,i]   via all-ones lhsT   (sum over partitions AND
                                                  broadcast to 128 parts)
    svT[c,i]= (sum_j V[j,c] E[j,i]) * recip[i]   (lhsT=V, rhs=E)
    y[c',n] = x[c',n] + sum_c woT[c,c'] svT[c,n] + bo_eff[c']
  with bo_eff = bo + wo @ bv (V bias folded in on the host).
"""

import math

import numpy as np

import concourse.mybir as mybir
import concourse.tile as tile
from concourse import bacc, bass_utils

B, C, H, W = 16, 512, 32, 32
N = H * W           # 1024 tokens
NCORES = 8
BPC = B // NCORES   # batch items per core
P = 128
CO = C // P         # 4 channel chunks
NB = N // 512       # 2 psum-bank slices of the token dim
NT = N // P         # 8 token chunks

_CACHE: dict = {}


def _build(reps: int = 1):
    f32 = mybir.dt.float32
    f32r = mybir.dt.float32r
    Ident = mybir.ActivationFunctionType.Identity
    Exp = mybir.ActivationFunctionType.Exp
    add = mybir.AluOpType.add

    nc = bacc.Bacc("TRN2", debug=False, enable_asserts=False, num_devices=NCORES)
    x_d = nc.dram_tensor("x", (BPC, C, N), f32r, kind="ExternalInput").ap()
    w_d = {
        k: nc.dram_tensor(f"w{k}t", (C, C), f32r, kind="ExternalInput").ap()
        for k in ("q", "k", "v", "o")
    }
    bq_d = nc.dram_tensor("bq", (P, CO), f32, kind="ExternalInput").ap()
    bk_d = nc.dram_tensor("bk", (P, CO), f32, kind="ExternalInput").ap()
    bo_d = nc.dram_tensor("bo", (P, CO), f32, kind="ExternalInput").ap()
    ones_d = nc.dram_tensor("ones", (P, P), f32r, kind="ExternalInput").ap()
    y_d = nc.dram_tensor("y", (BPC, C, N), f32, kind="ExternalOutput").ap()

    with tile.TileContext(nc) as tc:
        with (
            tc.tile_pool(name="wp", bufs=1) as wp,
            tc.tile_pool(name="xp", bufs=2) as xp,
            tc.tile_pool(name="qkp", bufs=1) as qkp,
            tc.tile_pool(name="vp", bufs=2) as vp,
            tc.tile_pool(name="ep", bufs=1) as ep,
            tc.tile_pool(name="svp", bufs=1) as svp,
            tc.tile_pool(name="rp", bufs=1) as rp,
            tc.tile_pool(name="yp", bufs=4) as yp,
            tc.tile_pool(name="ps", bufs=6, space="PSUM") as ps,
        ):
            # Per-ci chunked loads in first-consumption order, so the PE can
            # start on (wq ci-chunk 0, x ci-chunk 0) instead of waiting for
            # the full 6 MB weight+activation prefix.
            wt = {
                k: wp.tile([P, CO, C], f32r, tag=f"w{k}", name=f"w{k}")
                for k in ("q", "k", "v", "o")
            }
            x_tiles = [
                xp.tile([P, CO, N], f32r, tag="x", name=f"x{b}") for b in range(BPC)
            ]
            w_r = {
                k: w_d[k].rearrange("(co p) o -> p co o", p=P) for k in ("q", "k", "v", "o")
            }
            x_r = [x_d[b].rearrange("(ci p) n -> p ci n", p=P) for b in range(BPC)]
            # PE warm-up on a memset tile (no DMA dependency): keeps the HAM
            # activity window busy while the first wq/x chunks stream in.
            warm_t = wp.tile([P, P], f32, tag="warm_t")
            nc.vector.memset(warm_t[:], 0.0)
            with tc.tile_pool(name="warm", bufs=1, space="PSUM") as warmp:
                for i in range(8):
                    wpt = warmp.tile([P, P], f32, tag="warm", name=f"warm{i}")
                    nc.tensor.matmul(wpt[:], warm_t[:], warm_t[:],
                                     start=True, stop=True)
            # loads in first-consumption order
            for ci in range(CO):
                nc.sync.dma_start(wt["q"][:, ci], w_r["q"][:, ci])
                nc.sync.dma_start(x_tiles[0][:, ci, 0:512], x_r[0][:, ci, 0:512])
            bq_t = wp.tile([P, CO], f32, tag="bq")
            nc.sync.dma_start(bq_t[:], bq_d)
            bk_t = wp.tile([P, CO], f32, tag="bk")
            nc.sync.dma_start(bk_t[:], bk_d)
            for ci in range(CO):
                nc.sync.dma_start(wt["k"][:, ci], w_r["k"][:, ci])
            for ci in range(CO):
                nc.sync.dma_start(x_tiles[0][:, ci, 512:1024], x_r[0][:, ci, 512:1024])
            for ci in range(CO):
                nc.sync.dma_start(wt["v"][:, ci], w_r["v"][:, ci])
            ones_t = wp.tile([P, P], f32r, tag="ones")
            nc.sync.dma_start(ones_t[:], ones_d)
            for ci in range(CO):
                nc.sync.dma_start(wt["o"][:, ci], w_r["o"][:, ci])
            bo_t = wp.tile([P, CO], f32, tag="bo")
            nc.sync.dma_start(bo_t[:], bo_d)
            for ci in range(CO):
                nc.sync.dma_start(x_tiles[1][:, ci], x_r[1][:, ci])

            inv_sqrt_c = 1.0 / math.sqrt(C)

            for b in [i for _ in range(reps) for i in range(BPC)]:
                x_sb = x_tiles[b]

                # --- Qt / Kt projections (channel-major) ---
                qt = qkp.tile([P, CO, N], f32r, tag="qt")
                kt = qkp.tile([P, CO, N], f32r, tag="kt")
                for nb in range(NB):
                    for t_sb, w_t, b_t in ((qt, wt["q"], bq_t), (kt, wt["k"], bk_t)):
                        for oc in range(CO):
                            pt = ps.tile([P, 512], f32, tag="ps")
                            for ci in range(CO):
                                nc.tensor.matmul(
                                    pt[:],
                                    w_t[:, ci, oc * P:(oc + 1) * P],
                                    x_sb[:, ci, nb * 512:(nb + 1) * 512],
                                    start=(ci == 0), stop=(ci == CO - 1),
                                )
                            nc.scalar.activation(
                                t_sb[:, oc, nb * 512:(nb + 1) * 512], pt[:],
                                Ident, bias=b_t[:, oc:oc + 1],
                            )

                # --- St = Kt^T Qt, then exp (scores scaled inside ACT).
                #     DVE accumulates the softmax denominators chunk by
                #     chunk as the exps land (frees the PE). ---
                est = ep.tile([P, NT, N], f32r, tag="est")
                esum = rp.tile([P, N], f32r, tag="esum")
                for jc in range(NT):
                    for ib in range(NB):
                        pt = ps.tile([P, 512], f32, tag="ps")
                        for oc in range(CO):
                            nc.tensor.matmul(
                                pt[:],
                                kt[:, oc, jc * P:(jc + 1) * P],
                                qt[:, oc, ib * 512:(ib + 1) * 512],
                                start=(oc == 0), stop=(oc == CO - 1),
                            )
                        nc.scalar.activation(
                            est[:, jc, ib * 512:(ib + 1) * 512], pt[:],
                            Exp, scale=inv_sqrt_c,
                        )
                    if jc == 1:
                        nc.vector.tensor_add(esum[:], est[:, 0, :], est[:, 1, :])
                    elif jc > 1:
                        nc.vector.tensor_add(esum[:], esum[:], est[:, jc, :])

                # --- V projection (token-major, bias folded into bo_eff) ---
                v_sb = vp.tile([P, NT, C], f32r, tag="v")
                for t8 in range(NT):
                    pt = ps.tile([P, 512], f32, tag="ps")
                    for ci in range(CO):
                        nc.tensor.matmul(
                            pt[:],
                            x_sb[:, ci, t8 * P:(t8 + 1) * P],
                            wt["v"][:, ci, :],
                            start=(ci == 0), stop=(ci == CO - 1),
                        )
                    nc.vector.tensor_copy(v_sb[:, t8, :], pt[:])

                # --- svT = (V^T E) * recip  (channel-major). The partition
                #     sum+broadcast of esum (all-ones matmul) and reciprocal
                #     are slotted after the first group's matmuls so the PE
                #     never waits on the DVE chain. ---
                recip = rp.tile([P, N], f32, tag="recip")
                sv = svp.tile([P, CO, N], f32r, tag="sv")
                first = True
                for cc in range(CO):
                    for ib in range(NB):
                        pt = ps.tile([P, 512], f32, tag="ps")
                        for jc in range(NT):
                            nc.tensor.matmul(
                                pt[:],
                                v_sb[:, jc, cc * P:(cc + 1) * P],
                                est[:, jc, ib * 512:(ib + 1) * 512],
                                start=(jc == 0), stop=(jc == NT - 1),
                            )
                        if first:
                            first = False
                            for db in range(NB):
                                dpt = ps.tile([P, 512], f32, tag="ps")
                                nc.tensor.matmul(
                                    dpt[:], ones_t[:],
                                    esum[:, db * 512:(db + 1) * 512],
                                    start=True, stop=True,
                                )
                                nc.vector.reciprocal(
                                    recip[:, db * 512:(db + 1) * 512], dpt[:])
                        nc.vector.tensor_mul(
                            sv[:, cc, ib * 512:(ib + 1) * 512], pt[:],
                            recip[:, ib * 512:(ib + 1) * 512],
                        )

                # --- output projection + bias + residual ---
                for c2 in range(CO):
                    for nb in range(NB):
                        pt = ps.tile([P, 512], f32, tag="ps")
                        for cc in range(CO):
                            nc.tensor.matmul(
                                pt[:],
                                wt["o"][:, cc, c2 * P:(c2 + 1) * P],
                                sv[:, cc, nb * 512:(nb + 1) * 512],
                                start=(cc == 0), stop=(cc == CO - 1),
                            )
                        yt = yp.tile([P, 512], f32, tag="y")
                        nc.vector.scalar_tensor_tensor(
                            yt[:], pt[:], bo_t[:, c2:c2 + 1],
                            x_sb[:, c2, nb * 512:(nb + 1) * 512].bitcast(f32),
                            add, add,
                        )
                        nc.sync.dma_start(
                            y_d[b, c2 * P:(c2 + 1) * P, nb * 512:(nb + 1) * 512],
                            yt[:],
                        )
    nc.compile()
    return nc


def _prep_inputs(inputs):
    x = np.asarray(inputs["x"], np.float32).reshape(B, C, N)
    wts = {}
    for k in ("q", "k", "v", "o"):
        wts[f"w{k}t"] = np.ascontiguousarray(np.asarray(inputs[f"w{k}"], np.float32).T)
    bq = np.asarray(inputs["bq"], np.float32)
    bk = np.asarray(inputs["bk"], np.float32)
    bv = np.asarray(inputs["bv"], np.float32)
    bo = np.asarray(inputs["bo"], np.float32)
    wo = np.asarray(inputs["wo"], np.float32)
    bo_eff = bo + wo @ bv

    def per_part(v):  # [C] -> [P, CO]
        return np.ascontiguousarray(v.reshape(CO, P).T)

    shared = {
        **wts,
        "bq": per_part(bq),
        "bk": per_part(bk),
        "bo": per_part(bo_eff),
        "ones": np.ones((P, P), np.float32),
    }
    in_maps = [
        {**shared, "x": np.ascontiguousarray(x[i * BPC:(i + 1) * BPC])}
        for i in range(NCORES)
    ]
    return in_maps


def _make_axon_runner(nc):
    """Cached jitted shard_map runner for the axon/PJRT path, so repeated
    kernel() calls execute without re-tracing (the stock
    run_bass_kernel_spmd path builds a fresh jit closure per call)."""
    import jax
    from jax.sharding import Mesh, NamedSharding, PartitionSpec

    import warnings

    with warnings.catch_warnings():
        warnings.simplefilter("ignore")
        from jax.experimental.shard_map import shard_map

    import concourse.bass2jax as b2j

    b2j.install_neuronx_cc_hook()
    partition_name = nc.partition_id_tensor.name if nc.partition_id_tensor else None
    in_names, out_names, out_avals = [], [], []
    for alloc in nc.m.functions[0].allocations:
        if not isinstance(alloc, mybir.MemoryLocationSet):
            continue
        name = alloc.memorylocations[0].name
        if alloc.kind == "ExternalInput":
            if name != partition_name:
                in_names.append(name)
        elif alloc.kind == "ExternalOutput":
            out_names.append(name)
            out_avals.append(
                jax.core.ShapedArray(tuple(alloc.tensor_shape),
                                     mybir.dt.np(alloc.dtype)))
    n_params = len(in_names)
    bind_in_names = list(in_names) + list(out_names)
    if partition_name is not None:
        bind_in_names.append(partition_name)

    def _body(*args):
        operands = list(args)
        if partition_name is not None:
            operands.append(b2j.partition_id_tensor())
        return tuple(b2j._bass_exec_p.bind(
            *operands,
            out_avals=tuple(out_avals),
            in_names=tuple(bind_in_names),
            out_names=tuple(out_names),
            lowering_input_output_aliases=(),
            sim_require_finite=True,
            sim_require_nnan=True,
            nc=nc,
        ))

    devices = jax.devices()[:NCORES]
    mesh = Mesh(np.asarray(devices), ("core",))
    n_outs = len(out_avals)
    fn = jax.jit(
        shard_map(_body, mesh=mesh,
                  in_specs=(PartitionSpec("core"),) * (n_params + n_outs),
                  out_specs=(PartitionSpec("core"),) * n_outs,
                  check_rep=False),
        keep_unused=True,
    )
    sharding = NamedSharding(mesh, PartitionSpec("core"))
    zeros = [
        np.zeros((NCORES * a.shape[0], *a.shape[1:]), a.dtype) for a in out_avals
    ]
    dev_zeros = [jax.device_put(z, sharding) for z in zeros]

    def run(in_maps):
        concat_in = [
            np.concatenate([np.asarray(m[nm]) for m in in_maps], axis=0)
            for nm in in_names
        ]
        dev_in = [jax.device_put(a, sharding) for a in concat_in]
        outs = fn(*dev_in, *dev_zeros)
        return [
            {nm: np.asarray(outs[i]).reshape(NCORES, *out_avals[i].shape)[c]
             for i, nm in enumerate(out_names)}
            for c in range(NCORES)
        ]

    return run


def kernel(**inputs) -> np.ndarray:
    if "nc" not in _CACHE:
        _CACHE["nc"] = _build()
    nc = _CACHE["nc"]
    in_maps = _prep_inputs(inputs)

    from concourse._compat import axon_active

    if axon_active():
        if "runner" not in _CACHE:
            _CACHE["runner"] = _make_axon_runner(nc)
        results = _CACHE["runner"](in_maps)
    else:
        results = bass_utils.run_bass_kernel_spmd(
            nc, in_maps, core_ids=list(range(NCORES))).results
    y = np.concatenate([r["y"] for r in results], axis=0)
    return y.reshape(B, C, H, W)
